# revision 1
# baseline (speedup 1.0000x reference)
"""Trainium2 Bass kernel: 3x3 same-padding conv2d, NCHW.

Full inputs: x (32, 64, 112, 112) f32, W (64, 128, 3, 3) f32 (IOHW).
Full output: (32, 128, 112, 112) f32.

Strategy: data-parallel over batch across 8 NeuronCores (4 images/core).
Per core, images are processed as 2 pairs: partitions 0-63 hold the even
image's 64 input channels, partitions 64-127 the odd image's. The 3x3 conv
is 9 shift-offset matmuls (contraction over cin=64) accumulated in PSUM.
The two images run as independent K=64 matmul streams in PE row-groups
{0,1} and {2,3} (tile_position auto-derived from base partition), which the
PE executes concurrently, recovering full-array throughput. fp32r operands
give 1 cycle/row matmul speed at N=448.

Host side pre-pads images to 114x114 (zero border = conv padding) so every
DMA is fully contiguous and no on-chip memsets or edge fixups are needed.
"""

import numpy as np

import concourse.bacc as bacc
import concourse.tile as tile
import concourse.mybir as mybir
from concourse.bass_utils import run_bass_kernel_spmd

F32 = mybir.dt.float32
F32R = mybir.dt.float32r

NCORES = 8
H = W_ = 112
HP = H + 2  # padded
NTAP = 9
NTILE = H // 4  # 28 output-row tiles of 4 rows x 112 cols = 448
TAPS = [(u, v) for u in range(3) for v in range(3)]

_NC_CACHE = []


def _build():
    nc = bacc.Bacc()
    xp_ext = nc.declare_dram_parameter("xp", [2, 128, HP, HP], F32, isOutput=False)
    wt_ext = nc.declare_dram_parameter("wt", [128, NTAP * 128], F32, isOutput=False)
    out_ext = nc.declare_dram_parameter("out", [4, 128, H, W_], F32, isOutput=True)

    with tile.TileContext(nc) as tc:
        with (
            tc.tile_pool(name="xpool", bufs=2) as xpool,
            tc.tile_pool(name="wpool", bufs=1) as wpool,
            tc.tile_pool(name="opool", bufs=3) as opool,
            tc.tile_pool(name="psum", bufs=3, space="PSUM") as psum,
        ):
            wt = wpool.tile([128, NTAP * 128], F32R)
            nc.gpsimd.dma_start(out=wt[:], in_=wt_ext[:])

            for pair in range(2):
                xt = xpool.tile([128, HP, HP], F32R)
                # two chunks so load of pair 1 overlaps compute of pair 0
                nc.gpsimd.dma_start(
                    out=xt[:, 0:57, :], in_=xp_ext[pair, :, 0:57, :]
                )
                nc.gpsimd.dma_start(
                    out=xt[:, 57:HP, :], in_=xp_ext[pair, :, 57:HP, :]
                )

                for ti in range(NTILE):
                    r0 = 4 * ti
                    ps_l = psum.tile([128, 448], F32)
                    ps_h = psum.tile([128, 448], F32)
                    for t, (u, v) in enumerate(TAPS):
                        nc.tensor.matmul(
                            ps_l[:],
                            wt[0:64, t * 128 : (t + 1) * 128],
                            xt[0:64, r0 + u : r0 + u + 4, v : v + W_],
                            start=(t == 0),
                            stop=(t == NTAP - 1),
                        )
                        nc.tensor.matmul(
                            ps_h[:],
                            wt[64:128, t * 128 : (t + 1) * 128],
                            xt[64:128, r0 + u : r0 + u + 4, v : v + W_],
                            start=(t == 0),
                            stop=(t == NTAP - 1),
                        )
                    o_l = opool.tile([128, 448], F32)
                    o_h = opool.tile([128, 448], F32)
                    nc.vector.tensor_copy(o_l[:], ps_l[:])
                    nc.vector.tensor_copy(o_h[:], ps_h[:])
                    nc.sync.dma_start(
                        out=out_ext[2 * pair, :, r0 : r0 + 4, :], in_=o_l[:]
                    )
                    nc.sync.dma_start(
                        out=out_ext[2 * pair + 1, :, r0 : r0 + 4, :], in_=o_h[:]
                    )
    nc.finalize()
    return nc


def get_nc():
    if not _NC_CACHE:
        _NC_CACHE.append(_build())
    return _NC_CACHE[0]


def make_in_maps(x, W):
    x = np.ascontiguousarray(np.asarray(x, dtype=np.float32))
    W = np.ascontiguousarray(np.asarray(W, dtype=np.float32))
    # lhsT per tap t=(u,v): [cin, cout] = W[:, :, u, v]; layout (cin, tap, cout)
    wt_half = np.ascontiguousarray(W.transpose(0, 2, 3, 1)).reshape(64, NTAP * 128)
    wt = np.concatenate([wt_half, wt_half], axis=0)  # duplicate for both halves
    in_maps = []
    for c in range(NCORES):
        xs = x[c * 4 : (c + 1) * 4].reshape(2, 128, H, W_)
        xp = np.zeros((2, 128, HP, HP), dtype=np.float32)
        xp[:, :, 1 : H + 1, 1 : W_ + 1] = xs
        in_maps.append({"xp": xp, "wt": wt})
    return in_maps


def kernel(x, W):
    nc = get_nc()
    in_maps = make_in_maps(x, W)
    res = run_bass_kernel_spmd(nc, in_maps, list(range(NCORES)))
    out = np.concatenate([res.results[c]["out"] for c in range(NCORES)], axis=0)
    return out


# revision 2
# speedup vs baseline: 1.0031x; 1.0031x over previous
"""Trainium2 Bass kernel: 3x3 same-padding conv2d, NCHW.

Full inputs: x (32, 64, 112, 112) f32, W (64, 128, 3, 3) f32 (IOHW).
Full output: (32, 128, 112, 112) f32.

Strategy: data-parallel over batch across 8 NeuronCores (4 images/core).
Per core, images are processed as 2 pairs: partitions 0-63 hold the even
image's 64 input channels, partitions 64-127 the odd image's. The 3x3 conv
is 9 shift-offset matmuls (contraction over cin=64) accumulated in PSUM.
The two images run as independent K=64 matmul streams in PE row-groups
{0,1} and {2,3} (tile_position auto-derived from base partition), which the
PE executes concurrently, recovering full-array throughput. fp32r operands
give 1 cycle/row matmul speed at N=448.

Host side pre-pads images to 114x114 (zero border = conv padding) so every
DMA is fully contiguous and no on-chip memsets or edge fixups are needed.
"""

import numpy as np

import concourse.bacc as bacc
import concourse.tile as tile
import concourse.mybir as mybir
from concourse.bass_utils import run_bass_kernel_spmd

F32 = mybir.dt.float32
F32R = mybir.dt.float32r

NCORES = 8
H = W_ = 112
HP = H + 2  # padded
NTAP = 9
NTILE = H // 4  # 28 output-row tiles of 4 rows x 112 cols = 448
TAPS = [(u, v) for u in range(3) for v in range(3)]

_NC_CACHE = []


def _build():
    nc = bacc.Bacc()
    xp_ext = nc.declare_dram_parameter("xp", [2, 128, HP, HP], F32R, isOutput=False)
    wt_ext = nc.declare_dram_parameter("wt", [128, NTAP * 128], F32R, isOutput=False)
    out_ext = nc.declare_dram_parameter("out", [4, 128, H, W_], F32, isOutput=True)

    with tile.TileContext(nc) as tc:
        with (
            tc.tile_pool(name="xpool", bufs=2) as xpool,
            tc.tile_pool(name="wpool", bufs=1) as wpool,
            tc.tile_pool(name="opool", bufs=3) as opool,
            tc.tile_pool(name="psum", bufs=3, space="PSUM") as psum,
        ):
            wt = wpool.tile([128, NTAP * 128], F32R)
            nc.sync.dma_start(out=wt[:], in_=wt_ext[:])

            for pair in range(2):
                xt = xpool.tile([128, HP, HP], F32R)
                # two chunks so load of pair 1 overlaps compute of pair 0
                nc.sync.dma_start(
                    out=xt[:, 0:57, :], in_=xp_ext[pair, :, 0:57, :]
                )
                nc.sync.dma_start(
                    out=xt[:, 57:HP, :], in_=xp_ext[pair, :, 57:HP, :]
                )

                for ti in range(NTILE):
                    r0 = 4 * ti
                    ps_l = psum.tile([128, 448], F32)
                    ps_h = psum.tile([128, 448], F32)
                    for t, (u, v) in enumerate(TAPS):
                        nc.tensor.matmul(
                            ps_l[:],
                            wt[0:64, t * 128 : (t + 1) * 128],
                            xt[0:64, r0 + u : r0 + u + 4, v : v + W_],
                            start=(t == 0),
                            stop=(t == NTAP - 1),
                        )
                        nc.tensor.matmul(
                            ps_h[:],
                            wt[64:128, t * 128 : (t + 1) * 128],
                            xt[64:128, r0 + u : r0 + u + 4, v : v + W_],
                            start=(t == 0),
                            stop=(t == NTAP - 1),
                        )
                    o_l = opool.tile([128, 448], F32)
                    o_h = opool.tile([128, 448], F32)
                    nc.vector.tensor_copy(o_l[:], ps_l[:])
                    nc.vector.tensor_copy(o_h[:], ps_h[:])
                    nc.sync.dma_start(
                        out=out_ext[2 * pair, :, r0 : r0 + 4, :], in_=o_l[:]
                    )
                    nc.sync.dma_start(
                        out=out_ext[2 * pair + 1, :, r0 : r0 + 4, :], in_=o_h[:]
                    )
    nc.finalize()
    return nc


def get_nc():
    if not _NC_CACHE:
        _NC_CACHE.append(_build())
    return _NC_CACHE[0]


def make_in_maps(x, W):
    x = np.ascontiguousarray(np.asarray(x, dtype=np.float32))
    W = np.ascontiguousarray(np.asarray(W, dtype=np.float32))
    # lhsT per tap t=(u,v): [cin, cout] = W[:, :, u, v]; layout (cin, tap, cout)
    wt_half = np.ascontiguousarray(W.transpose(0, 2, 3, 1)).reshape(64, NTAP * 128)
    wt = np.concatenate([wt_half, wt_half], axis=0)  # duplicate for both halves
    in_maps = []
    for c in range(NCORES):
        xs = x[c * 4 : (c + 1) * 4].reshape(2, 128, H, W_)
        xp = np.zeros((2, 128, HP, HP), dtype=np.float32)
        xp[:, :, 1 : H + 1, 1 : W_ + 1] = xs
        in_maps.append({"xp": xp, "wt": wt})
    return in_maps


def kernel(x, W):
    nc = get_nc()
    in_maps = make_in_maps(x, W)
    res = run_bass_kernel_spmd(nc, in_maps, list(range(NCORES)))
    out = np.concatenate([res.results[c]["out"] for c in range(NCORES)], axis=0)
    return out
